# revision 1
# baseline (speedup 1.0000x reference)
"""AttnBlock (GroupNorm -> QKV 1x1 -> single-head attention over 4096 tokens
-> proj -> residual) on 8 Trainium2 NeuronCores, data-parallel over batch.

v2: fp8e4m3 + DoubleRow matmuls everywhere that matters, everything resident
in SBUF (no q/v DRAM spills).

Per-core layout (one image per core, N=4096 tokens, C=512 channels):
  - x streamed token-major in chunks, converted to fp8; GroupNorm stats via
    fp8 DoubleRow ones-colsum matmuls; x PE-transposed in fp8 to hT
    [128, CP=2, 2, N] (c-tile pairs for DoubleRow contraction).
  - GroupNorm folded into weights: w8 = fp8(a_c * w * SW), effective biases
    b_c @ w + b computed with f32r matmuls on the raw weights.  SW=16 weight
    prescale keeps fp8 weights well inside normal range.
  - q^T, k^T computed channel-major via DR matmuls, stored fp8 true-scale
    (scale 1/SW + bias in the PSUM->SBUF copy); v token-major, stored fp8 at
    SW scale, all resident: q8T/k8T [128, CP, 2, N], v8 [128, JP=16, 2, C].
  - Attention per query group of 512: scores via DR (contraction 2x128 chans),
    exp on ACT with range shift -SHIFT (cancels in softmax) directly to fp8
    a2 [128, 2, GW]; AV + denominator accumulate over 16 key-tile pairs with
    DR matmuls.  avT copied to fp8 at true scale (1/SW); denominator
    reciprocal (folded 1/SW for the wo scale) bounced via DRAM to
    per-partition layout, consumed only by the final DVE residual-add, so the
    tensor engine never waits on it.  proj for group g is issued during group
    g+1's score matmuls to hide the avT copy + bounce latency.
"""

import numpy as np
from collections import deque

import concourse.bass as bass
import concourse.tile as tile
from concourse import bacc, mybir
from concourse.bass_utils import run_bass_kernel_spmd

B, H, W, C = 8, 64, 64, 512
N = H * W            # 4096 tokens per image
G = 32               # groups
EPS = 1e-5
N_CORES = 8

F32 = mybir.dt.float32
F32R = mybir.dt.float32r
FP8 = mybir.dt.float8e4
BF16 = mybir.dt.bfloat16
AF = mybir.ActivationFunctionType
ALU = mybir.AluOpType
DR = mybir.MatmulPerfMode.DoubleRow

NT = N // 128         # 32 token tiles
CT = C // 128         # 4 channel tiles
CP = CT // 2          # 2 channel-tile pairs (DoubleRow)
JP = NT // 2          # 16 key-tile pairs
NG = 8                # query groups
GW = N // NG          # 512 queries per group
NB = GW // 128        # 4 token blocks per group
CPG = C // G          # 16 channels per group

SW = 16.0             # weight prescale (exact power of 2)
SHIFT = 3.0           # exp range shift; cancels in softmax
XA = 32.0             # extra avT downscale: keeps fp8 avT under the 240 max
SCALE = float(C) ** -0.5


def build_program(reps: int = 1, dump: bool = False):
    nc = bacc.Bacc("TRN2", target_bir_lowering=False, debug=False,
                   num_devices=N_CORES)

    x_ap = nc.dram_tensor("x", [N, C], F32, kind="ExternalInput").ap()
    wq_ap = nc.dram_tensor("wq", [C, C], F32, kind="ExternalInput").ap()
    wk_ap = nc.dram_tensor("wk", [C, C], F32, kind="ExternalInput").ap()
    wv_ap = nc.dram_tensor("wv", [C, C], F32, kind="ExternalInput").ap()
    wo_ap = nc.dram_tensor("wo", [C, C], F32, kind="ExternalInput").ap()
    bq_ap = nc.dram_tensor("bq", [C], F32, kind="ExternalInput").ap()
    bk_ap = nc.dram_tensor("bk", [C], F32, kind="ExternalInput").ap()
    bv_ap = nc.dram_tensor("bv", [C], F32, kind="ExternalInput").ap()
    bo_ap = nc.dram_tensor("bo", [C], F32, kind="ExternalInput").ap()
    gns_ap = nc.dram_tensor("gn_scale", [C], F32, kind="ExternalInput").ap()
    gnb_ap = nc.dram_tensor("gn_bias", [C], F32, kind="ExternalInput").ap()
    id_ap = nc.dram_tensor("ident", [128, 128], F32, kind="ExternalInput").ap()
    out_ap = nc.dram_tensor("out", [N, C], F32, kind="ExternalOutput").ap()

    dbg = {}
    if dump == 4:   # full phase-C diagnosis
        dbg["avT_all"] = nc.dram_tensor("dbg_avT_all", [NG, 128, CP, 2, GW],
                                        FP8, kind="ExternalOutput").ap()
        dbg["o_all"] = nc.dram_tensor("dbg_o_all", [NG, NB, 128, C], F32,
                                      kind="ExternalOutput").ap()
        dbg["xb_all"] = nc.dram_tensor("dbg_xb_all", [NG, NB, 128, C], F32,
                                       kind="ExternalOutput").ap()
        dbg["r_all4"] = nc.dram_tensor("dbg_r_all4", [NG, 128, NB], F32,
                                       kind="ExternalOutput").ap()
    if dump == 3:   # r-path diagnosis
        dbg["r_all"] = nc.dram_tensor("dbg_r_all", [NG, 128, NB], F32,
                                      kind="ExternalOutput").ap()
        dbg["dT_all"] = nc.dram_tensor("dbg_dT_all", [NG, 128, NB], F32,
                                       kind="ExternalOutput").ap()
        dbg["ds_all"] = nc.dram_tensor("dbg_ds_all", [NG, GW], F32,
                                       kind="ExternalOutput").ap()
    if dump == 2:   # zero/low-perturbation diagnosis
        for nm, shape, dt in (
            ("q8T_e", [128, CP, 2, N], FP8), ("k8T_e", [128, CP, 2, N], FP8),
            ("v8_e", [128, JP, 2, C], FP8)):
            dbg[nm] = nc.dram_tensor(f"dbg_{nm}", shape, dt,
                                     kind="ExternalOutput").ap()
    if dump is True:
        for nm, shape, dt in (
            ("hT", [128, CP, 2, N], FP8), ("q8T", [128, CP, 2, N], FP8),
            ("k8T", [128, CP, 2, N], FP8), ("v8", [128, JP, 2, C], FP8),
            ("w8q", [128, CP, 2, C], FP8), ("w8v", [128, CP, 2, C], FP8),
            ("s0", [128, 2, GW], F32), ("a20", [128, 2, GW], FP8),
            ("den0", [1, GW], F32), ("avT0", [128, CP, 2, GW], FP8),
            ("acbc", [2, C], F32), ("bqkT_d", [128, 2, CT], F32),
            ("bvb_d", [128, C], F32)):
            dbg[nm] = nc.dram_tensor(f"dbg_{nm}", shape, dt,
                                     kind="ExternalOutput").ap()
    gn_bounce = nc.dram_tensor("gn_bounce", [2, C], F32).ap()
    bias_bounce = nc.dram_tensor("bias_bounce", [3, C], F32).ap()
    den_bounce = nc.dram_tensor(
        "den_bounce", [NG, GW], F32,
        kind="ExternalOutput" if dump == 2 else "Internal").ap()

    x_r = x_ap.rearrange("(nt p) c -> nt p c", p=128)
    out_r = out_ap.rearrange("(nt p) c -> nt p c", p=128)

    with tile.TileContext(nc) as tc, \
         nc.allow_low_precision(reason="fp8 attention by design"):
        rep_ctx = tc.For_i(0, reps, 1) if reps > 1 else None
        import contextlib
        with contextlib.ExitStack() as st:
            if rep_ctx is not None:
                st.enter_context(rep_ctx)
            consts = st.enter_context(tc.tile_pool(name="consts", bufs=1))
            resid = st.enter_context(tc.tile_pool(name="resid", bufs=1))

            # ---- constants -------------------------------------------------
            id_raw = consts.tile([128, 128], F32, tag="id_raw")
            nc.scalar.dma_start(id_raw[:], id_ap[:])
            id_bf = consts.tile([128, 128], BF16, tag="id_bf")
            nc.vector.tensor_copy(id_bf[:], id_raw[:])
            ones_f = consts.tile([128, 2, 1], F32, tag="ones_f")
            nc.vector.memset(ones_f[:], 1.0)
            ones8 = consts.tile([128, 2, 128], FP8, tag="ones8")
            nc.vector.tensor_copy(
                ones8[:], ones_f[:].broadcast_to([128, 2, 128]))
            ones_bf = consts.tile([128, 1], BF16, tag="ones_bf")
            nc.vector.tensor_copy(ones_bf[:], ones_f[:, 0, :])
            shift_t = consts.tile([128, 1], F32, tag="shift_t")
            nc.vector.memset(shift_t[:], -SHIFT)

            # raw weights in bf16 (bias matmuls + fp8 conversion source)
            w_r = {}
            with tc.tile_pool(name="wstage", bufs=2) as wstage:
                for name, ap in (("wq", wq_ap), ("wk", wk_ap), ("wv", wv_ap),
                                 ("wo", wo_ap)):
                    stg = wstage.tile([128, CT, C], F32, tag="wstg")
                    nc.scalar.dma_start(stg[:],
                                        ap.rearrange("(ct p) d -> p ct d", p=128))
                    raw = consts.tile([128, CT, C], BF16, tag=f"{name}_raw",
                                      name=f"{name}_raw")
                    nc.vector.tensor_copy(raw[:], stg[:])
                    w_r[name] = raw

            bqk_direct = consts.tile([128, 2, CT], F32, tag="bqk_direct")
            nc.scalar.dma_start(bqk_direct[:, 0, :],
                              bq_ap.rearrange("(dt p) -> p dt", p=128))
            nc.scalar.dma_start(bqk_direct[:, 1, :],
                              bk_ap.rearrange("(dt p) -> p dt", p=128))
            bv_direct = consts.tile([128, C], F32, tag="bv_direct")
            nc.scalar.dma_start(bv_direct[:],
                              bv_ap.unsqueeze(0).partition_broadcast(128))
            bqkT = consts.tile([128, 2, CT], F32, tag="bqkT")
            bvb = consts.tile([128, C], F32, tag="bvb")
            bob = consts.tile([128, C], F32, tag="bob")
            nc.scalar.dma_start(bob[:], bo_ap.unsqueeze(0).partition_broadcast(128))
            gns_sb = consts.tile([1, C], F32, tag="gns")
            nc.scalar.dma_start(gns_sb[:], gns_ap.unsqueeze(0))
            gnb_sb = consts.tile([1, C], F32, tag="gnb")
            nc.scalar.dma_start(gnb_sb[:], gnb_ap.unsqueeze(0))

            # fp8 weights, DoubleRow layout [128, CP, 2, C]
            w8 = {name: consts.tile([128, CP, 2, C], FP8, tag=f"{name}8",
                                    name=f"{name}8")
                  for name in ("wq", "wk", "wv", "wo")}

            # resident fp8 activations (+ bf16 x for the residual)
            x_bf = resid.tile([128, NT, C], BF16, tag="x_bf")
            q8T = resid.tile([128, CP, 2, N], FP8, tag="q8T")
            k8T = resid.tile([128, CP, 2, N], FP8, tag="k8T")
            v8 = resid.tile([128, JP, 2, C], FP8, tag="v8")

            with tc.tile_pool(name="hTp", bufs=1) as hTp, \
                 tc.tile_pool(name="small", bufs=1) as small:
                hT = hTp.tile([128, CP, 2, N], FP8, tag="hT")

                # ---- phase A: load x, stats, fp8 transpose -----------------
                CH = 4  # token tiles per chunk
                with (
                    tc.tile_pool(name="pa_ps", bufs=1, space=bass.MemorySpace.PSUM) as paps,
                    tc.tile_pool(name="pa_tps", bufs=4, space=bass.MemorySpace.PSUM) as patps,
                    tc.tile_pool(name="xin", bufs=3) as xin,
                    tc.tile_pool(name="x8p", bufs=2) as x8p,
                    tc.tile_pool(name="x28p", bufs=2) as x28p,
                ):
                    s1_ps = paps.tile([1, C], F32, tag="s1")
                    s2_ps = paps.tile([1, C], F32, tag="s2")
                    for ch in range(NT // CH):
                        x_t = xin.tile([128, CH, C], F32, tag="x_t")
                        nc.sync.dma_start(
                            x_t[:], x_r[ch * CH:(ch + 1) * CH].transpose([1, 0, 2]))
                        x8_t = x_bf[:, ch * CH:(ch + 1) * CH, :]
                        nc.vector.tensor_copy(x8_t, x_t[:])
                        x28_t = x28p.tile([128, CH, C], BF16, tag="x28_t")
                        nc.scalar.activation(x28_t[:], x8_t, AF.Square)
                        for u in range(CH):
                            nt = ch * CH + u
                            nc.tensor.matmul(s1_ps[:], ones_bf[:],
                                             x8_t[:, u, :],
                                             start=(nt == 0), stop=(nt == NT - 1))
                            nc.tensor.matmul(s2_ps[:], ones_bf[:],
                                             x28_t[:, u, :],
                                             start=(nt == 0), stop=(nt == NT - 1))
                        for ct in range(CT):
                            tp = patps.tile([128, 512], BF16, tag="tp")
                            for u in range(CH):
                                nc.tensor.transpose(
                                    tp[:, bass.ts(u, 128)],
                                    x8_t[:, u, bass.ts(ct, 128)], id_bf[:])
                            nc.vector.tensor_copy(
                                hT[:, ct // 2, ct % 2, bass.ts(ch, 512)], tp[:])

                    # group stats on partition 0
                    g1 = small.tile([1, G], F32, tag="g1")
                    nc.vector.reduce_sum(
                        g1[:], s1_ps[:].rearrange("p (g k) -> p g k", k=CPG),
                        axis=mybir.AxisListType.X)
                    g2 = small.tile([1, G], F32, tag="g2")
                    nc.vector.reduce_sum(
                        g2[:], s2_ps[:].rearrange("p (g k) -> p g k", k=CPG),
                        axis=mybir.AxisListType.X)
                    cnt = 1.0 / (N * CPG)
                    mean = small.tile([1, G], F32, tag="mean")
                    nc.scalar.mul(mean[:], g1[:], cnt)
                    ex2 = small.tile([1, G], F32, tag="ex2")
                    nc.scalar.mul(ex2[:], g2[:], cnt)
                    var = small.tile([1, G], F32, tag="var")
                    nc.vector.tensor_tensor(var[:], mean[:], mean[:], op=ALU.mult)
                    nc.vector.tensor_tensor(var[:], ex2[:], var[:], op=ALU.subtract)
                    eps_t = small.tile([1, 1], F32, tag="eps_t")
                    nc.vector.memset(eps_t[:], EPS)
                    sd = small.tile([1, G], F32, tag="sd")
                    nc.scalar.activation(sd[:], var[:], AF.Sqrt, bias=eps_t[:])
                    inv = small.tile([1, G], F32, tag="inv")
                    nc.vector.reciprocal(inv[:], sd[:])
                    invc = small.tile([1, C], F32, tag="invc")
                    nc.vector.tensor_copy(
                        invc[:].rearrange("p (g k) -> p g k", k=CPG),
                        inv[:].unsqueeze(2).broadcast_to([1, G, CPG]))
                    meanc = small.tile([1, C], F32, tag="meanc")
                    nc.vector.tensor_copy(
                        meanc[:].rearrange("p (g k) -> p g k", k=CPG),
                        mean[:].unsqueeze(2).broadcast_to([1, G, CPG]))
                    a_c = small.tile([1, C], F32, tag="a_c")
                    nc.vector.tensor_tensor(a_c[:], invc[:], gns_sb[:], op=ALU.mult)
                    b_c = small.tile([1, C], F32, tag="b_c")
                    nc.vector.tensor_tensor(b_c[:], meanc[:], a_c[:], op=ALU.mult)
                    nc.vector.tensor_tensor(b_c[:], gnb_sb[:], b_c[:], op=ALU.subtract)
                    a_c_sw = small.tile([1, C], F32, tag="a_c_sw")
                    nc.scalar.mul(a_c_sw[:], a_c[:], SW)
                    # bounce [1, C] -> per-partition [128, 2, CT]
                    nc.sync.dma_start(gn_bounce[0].unsqueeze(0), a_c_sw[:])
                    nc.sync.dma_start(gn_bounce[1].unsqueeze(0), b_c[:])
                    ab_sb = small.tile([128, 2, CT], F32, tag="ab_sb")
                    nc.scalar.dma_start(
                        ab_sb[:], gn_bounce.rearrange("two (ct p) -> p two ct", p=128))
                    b_rT = small.tile([128, CT], BF16, tag="b_rT")
                    nc.vector.tensor_copy(b_rT[:], ab_sb[:, 1, :])

                    # effective biases: b_c @ w + orig_bias (bf16)
                    with tc.tile_pool(name="bps", bufs=1,
                                      space=bass.MemorySpace.PSUM) as bps:
                        for i, name in enumerate(("wq", "wk", "wv")):
                            bp = bps.tile([1, C], F32, tag="bp", name=f"bp{i}")
                            for ct in range(CT):
                                nc.tensor.matmul(
                                    bp[:], b_rT[:, ct:ct + 1], w_r[name][:, ct, :],
                                    start=(ct == 0), stop=(ct == CT - 1))
                            btmp = small.tile([1, C], F32, tag="btmp",
                                              name=f"btmp{i}")
                            nc.vector.tensor_copy(btmp[:], bp[:])
                            nc.sync.dma_start(bias_bounce[i].unsqueeze(0), btmp[:])
                            if name == "wv":
                                nc.scalar.dma_start(
                                    bvb[:],
                                    bias_bounce[i].unsqueeze(0).partition_broadcast(128))
                                nc.vector.tensor_tensor(bvb[:], bvb[:],
                                                        bv_direct[:], op=ALU.add)
                                # v is stored at SW scale -> bias scaled too
                                nc.vector.tensor_scalar(
                                    bvb[:], bvb[:], SW, 0.0,
                                    op0=ALU.mult, op1=ALU.add)
                            else:
                                nc.scalar.dma_start(
                                    bqkT[:, i, :],
                                    bias_bounce[i].rearrange("(dt p) -> p dt", p=128))
                                nc.vector.tensor_tensor(
                                    bqkT[:, i, :], bqkT[:, i, :],
                                    bqk_direct[:, i, :], op=ALU.add)
                        # fp8 weights: qkv get per-partition a_c*SW, wo gets SW
                        for name in ("wq", "wk", "wv"):
                            for ct in range(CT):
                                nc.scalar.activation(
                                    w8[name][:, ct // 2, ct % 2, :],
                                    w_r[name][:, ct, :],
                                    AF.Copy, bias=0.0,
                                    scale=ab_sb[:, 0, ct:ct + 1])
                        for ct in range(CT):
                            nc.scalar.activation(
                                w8["wo"][:, ct // 2, ct % 2, :],
                                w_r["wo"][:, ct, :],
                                AF.Copy, bias=0.0, scale=SW)

                # ---- phase B: q^T, k^T (channel-major), v (token-major) ----
                with (
                    tc.tile_pool(name="pb_ps", bufs=4, space=bass.MemorySpace.PSUM) as pbps,
                ):
                    for cb in range(CT * (N // 512)):
                        q_ps = pbps.tile([128, 512], F32, tag="qkv_ps")
                        k_ps = pbps.tile([128, 512], F32, tag="qkv_ps")
                        dt = cb % CT  # interleave dt and col blocks
                        nb8 = cb // CT
                        for cp in range(CP):
                            nc.tensor.matmul(
                                q_ps[:],
                                w8["wq"][:, cp, :, bass.ts(dt, 128)],
                                hT[:, cp, :, bass.ts(nb8, 512)],
                                start=(cp == 0), stop=(cp == CP - 1),
                                perf_mode=DR)
                        for cp in range(CP):
                            nc.tensor.matmul(
                                k_ps[:],
                                w8["wk"][:, cp, :, bass.ts(dt, 128)],
                                hT[:, cp, :, bass.ts(nb8, 512)],
                                start=(cp == 0), stop=(cp == CP - 1),
                                perf_mode=DR)
                        nc.scalar.activation(
                            q8T[:, dt // 2, dt % 2, bass.ts(nb8, 512)], q_ps[:],
                            AF.Identity, scale=1.0 / SW,
                            bias=bqkT[:, 0, dt:dt + 1])
                        nc.vector.tensor_scalar(
                            k8T[:, dt // 2, dt % 2, bass.ts(nb8, 512)], k_ps[:],
                            1.0 / SW, bqkT[:, 1, dt:dt + 1],
                            op0=ALU.mult, op1=ALU.add)
                    for nt in range(NT):
                        v_ps = pbps.tile([128, 512], F32, tag="qkv_ps")
                        for cp in range(CP):
                            nc.tensor.matmul(
                                v_ps[:],
                                hT[:, cp, :, bass.ts(nt, 128)],
                                w8["wv"][:, cp, :, :],
                                start=(cp == 0), stop=(cp == CP - 1),
                                perf_mode=DR)
                        nc.vector.tensor_tensor(
                            v8[:, nt // 2, nt % 2, :], v_ps[:], bvb[:], op=ALU.add)

            if dump is True:
                nc.sync.dma_start(dbg["hT"][:], hT[:])
                nc.sync.dma_start(dbg["q8T"][:], q8T[:])
                nc.sync.dma_start(dbg["k8T"][:], k8T[:])
                nc.sync.dma_start(dbg["v8"][:], v8[:])
                nc.sync.dma_start(dbg["w8q"][:], w8["wq"][:])
                nc.sync.dma_start(dbg["w8v"][:], w8["wv"][:])
                nc.sync.dma_start(dbg["acbc"][0].unsqueeze(0), a_c_sw[:])
                nc.sync.dma_start(dbg["acbc"][1].unsqueeze(0), b_c[:])
                nc.sync.dma_start(dbg["bqkT_d"][:], bqkT[:])
                nc.sync.dma_start(dbg["bvb_d"][:], bvb[:])

            # ---- phase C: attention + proj + residual ----------------------
            with (
                tc.tile_pool(name="pc_s", bufs=2, space=bass.MemorySpace.PSUM) as pcs,
                tc.tile_pool(name="pc_av", bufs=4, space=bass.MemorySpace.PSUM) as pcav,
                tc.tile_pool(name="pc_o", bufs=2, space=bass.MemorySpace.PSUM) as pco,
                tc.tile_pool(name="atp", bufs=4) as atp,
                tc.tile_pool(name="avtp", bufs=2) as avtp,
                tc.tile_pool(name="xbp", bufs=4) as xbp,
                tc.tile_pool(name="obp", bufs=4) as obp,
                tc.tile_pool(name="rp", bufs=2) as rp,
            ):
                prev = None  # (g, avT, den_sb) pending proj

                def issue_proj(pg, avT, den_sb):
                    # den row -> per-partition reciprocal, all on-chip
                    denT = pco.tile([128, NB], F32, tag="o_ps",
                                    name=f"denT{pg}")
                    for nb in range(NB):
                        nc.tensor.transpose(denT[:, nb:nb + 1],
                                            den_sb[:, bass.ts(nb, 128)],
                                            id_raw[0:1, 0:1])
                    r_sb = rp.tile([128, NB], F32, tag="r_sb")
                    nc.vector.reciprocal(r_sb[:], denT[:])
                    if dump == 4:
                        nc.sync.dma_start(dbg["r_all4"][pg], r_sb[:])
                        nc.sync.dma_start(dbg["avT_all"][pg], avT[:])
                    if dump == 3:
                        dTs = rp.tile([128, NB], F32, tag="dTs", name=f"dTs{pg}")
                        nc.vector.tensor_copy(dTs[:], denT[:])
                        nc.sync.dma_start(dbg["dT_all"][pg], dTs[:])
                        nc.sync.dma_start(dbg["r_all"][pg], r_sb[:])
                        nc.sync.dma_start(dbg["ds_all"][pg].unsqueeze(0),
                                          den_sb[:])
                    for nb in range(NB):
                        o_ps = pco.tile([128, C], F32, tag="o_ps")
                        for cp in range(CP):
                            nc.tensor.matmul(
                                o_ps[:],
                                avT[:, cp, :, bass.ts(nb, 128)],
                                w8["wo"][:, cp, :, :],
                                start=(cp == 0), stop=(cp == CP - 1),
                                perf_mode=DR)
                        nt = pg * NB + nb
                        xb = xbp.tile([128, C], F32, tag="xb")
                        nc.vector.tensor_tensor(xb[:], x_bf[:, nt, :], bob[:],
                                                op=ALU.add)
                        o_sb = obp.tile([128, C], F32, tag="ob", name=f"osb{nb}")
                        nc.vector.tensor_tensor(
                            o_sb[:], o_ps[:],
                            r_sb[:, nb:nb + 1].broadcast_to([128, C]),
                            op=ALU.mult)
                        ob = obp.tile([128, C], F32, tag="ob")
                        nc.vector.tensor_tensor(ob[:], o_sb[:], xb[:],
                                                op=ALU.add)
                        nc.sync.dma_start(out_r[nt], ob[:])
                        if dump == 4:
                            nc.sync.dma_start(dbg["o_all"][pg, nb], o_sb[:])
                            nc.sync.dma_start(dbg["xb_all"][pg, nb], xb[:])

                for g in range(NG):
                    av_ps = [pcav.tile([128, GW], F32, tag="av",
                                       name=f"av_ps{g}_{dt}")
                             for dt in range(CT)]
                    den_ps = None  # allocated lazily, after prev group's proj
                    pend = deque()  # a2 tiles awaiting AV/den issue
                    for jp in range(JP):
                        a2 = atp.tile([128, 2, GW], FP8, tag="a2")
                        for i in range(2):
                            jt = jp * 2 + i
                            s_ps = pcs.tile([128, GW], F32, tag="s_ps")
                            for cp in range(CP):
                                nc.tensor.matmul(
                                    s_ps[:],
                                    k8T[:, cp, :, bass.ts(jt, 128)],
                                    q8T[:, cp, :, bass.ts(g, GW)],
                                    start=(cp == 0), stop=(cp == CP - 1),
                                    perf_mode=DR)
                            if dump is True and g == 0 and jp == 0:
                                sdmp = atp.tile([128, GW], F32, tag="sdmp",
                                                name=f"sdmp{i}")
                                nc.vector.tensor_copy(sdmp[:], s_ps[:])
                                nc.sync.dma_start(dbg["s0"][:, i, :], sdmp[:])
                            nc.scalar.activation(a2[:, i, :], s_ps[:], AF.Exp,
                                                 scale=SCALE, bias=shift_t[:])
                            if dump is True and g == 0 and jp == 0:
                                nc.sync.dma_start(dbg["a20"][:, i, :], a2[:, i, :])
                        pend.append((a2, jp))
                        if jp == 1 and prev is not None:
                            issue_proj(*prev)
                            prev = None
                        if len(pend) > 1:
                            if den_ps is None:
                                den_ps = pco.tile([128, GW], F32, tag="o_ps",
                                                  name=f"den{g}")
                            pa, pj = pend.popleft()
                            nc.tensor.matmul(den_ps[:], ones8[:], pa[:],
                                             start=(pj == 0), stop=False,
                                             perf_mode=DR)
                            for dt in range(CT):
                                nc.tensor.matmul(
                                    av_ps[dt][:],
                                    v8[:, pj, :, bass.ts(dt, 128)],
                                    pa[:],
                                    start=(pj == 0), stop=False,
                                    perf_mode=DR)
                    while pend:
                        pa, pj = pend.popleft()
                        last = not pend
                        nc.tensor.matmul(den_ps[:], ones8[:], pa[:],
                                         start=(pj == 0), stop=last,
                                         perf_mode=DR)
                        for dt in range(CT):
                            nc.tensor.matmul(
                                av_ps[dt][:],
                                v8[:, pj, :, bass.ts(dt, 128)],
                                pa[:],
                                start=(pj == 0), stop=last,
                                perf_mode=DR)
                    den_sb = rp.tile([1, GW], F32, tag="den_sb")
                    nc.scalar.activation(den_sb[:], den_ps[0:1, :], AF.Copy,
                                         bias=0.0, scale=SW / XA)
                    # AV^T -> fp8 SBUF at true scale (1/SW)
                    avT = avtp.tile([128, CP, 2, GW], FP8, tag="avT")
                    for dt in range(CT):
                        nc.vector.tensor_scalar(
                            avT[:, dt // 2, dt % 2, :], av_ps[dt][:],
                            1.0 / (SW * XA), 0.0, op0=ALU.mult, op1=ALU.add)
                    if dump is True and g == 0:
                        nc.sync.dma_start(dbg["avT0"][:], avT[:])
                    prev = (g, avT, den_sb)
                issue_proj(*prev)
                if dump == 2:
                    nc.sync.dma_start(dbg["q8T_e"][:], q8T[:])
                    nc.sync.dma_start(dbg["k8T_e"][:], k8T[:])
                    nc.sync.dma_start(dbg["v8_e"][:], v8[:])

    nc.compile()
    return nc


_CACHE = {}


def _get_program(reps: int = 1, dump: bool = False):
    key = (reps, dump)
    if key not in _CACHE:
        _CACHE[key] = build_program(reps, dump)
    return _CACHE[key]


def make_in_maps(inputs):
    ident = np.eye(128, dtype=np.float32)
    x = np.asarray(inputs["x"], dtype=np.float32).reshape(B, N, C)
    shared = {k: np.ascontiguousarray(np.asarray(inputs[k], dtype=np.float32))
              for k in ("wq", "wk", "wv", "wo", "bq", "bk", "bv", "bo",
                        "gn_scale", "gn_bias")}
    return [dict(x=np.ascontiguousarray(x[c]), ident=ident, **shared)
            for c in range(N_CORES)]


def kernel(**inputs) -> np.ndarray:
    nc = _get_program()
    in_maps = make_in_maps(inputs)
    last_err = None
    for _attempt in range(3):
        try:
            res = run_bass_kernel_spmd(nc, in_maps, list(range(N_CORES)))
            break
        except Exception as e:  # transient NRT device errors recover on retry
            last_err = e
    else:
        raise last_err
    out = np.stack([res.results[c]["out"] for c in range(N_CORES)], axis=0)
    return out.reshape(B, H, W, C)

